# revision 38
# baseline (speedup 1.0000x reference)
"""Multi-head attention (AttnProcessor) Bass kernel for 8 Trainium2 cores.

Problem: hidden_states [2,2048,1280], Wq/Wk/Wv/Wo [1280,1280], bo [1280],
20 heads x head_dim 64.  out = softmax(q k^T / 8) v @ Wo + bo.

Sharding: 40 (batch, head) units -> 8 cores x 5 heads.  Cores 0-3 take
batch 0, cores 4-7 batch 1; each core gets a 5-head column slice of
Wq/Wk/Wv and the matching row slice of Wo, computes its partial output
projection [S, D], and the host sums the 4 partials per batch and adds bo.

Per-core design (the HAM clock gate drops the PE to 1.2 GHz whenever the
PE idles, so everything aims at a gap-free PE instruction stream):

  phase 1 (dense matmuls, no cross-engine deps):
    kT for the full sequence, v for the full sequence, qT for qi chunk 0.
  phase 2 (attention):
    Heads are processed in row-tiled PAIRS: head 2p lives in partitions
    0:64 of qT/kT tile p, head 2p+1 in 64:128.  Emitting both heads' QK
    for one key tile back-to-back makes the PE run them concurrently in
    different row-groups of the array (tile_position auto-derives from
    the base partition), so a K=64 QK costs the same as K=128.
    Per (pair, kj) unit: QK of the NEXT unit -> filler matmuls -> exp of
    this unit on ScalarE ([128, 1024] covering both heads) -> both PV
    matmuls.  The 5th head runs solo with two kj tiles per unit.
    Filler = qT projections for later chunks + the previous chunk's
    output projection, so the PE always has independent queued work and
    never waits on the ScalarE exp latency.
  scores are computed transposed (S^T = kT-slice x qT, K=hd) so PV needs
    no transpose and the ones-augmented V gives the softmax denominator
    in the same PSUM accumulation chain.
  normalization runs off the PE: DVE copies (o_un + denominator row) free
    the PSUM bank ~1us after the pair ends, then DVE
    reciprocal_approx_fast + GpSimd partition_broadcast + DVE multiply.
    Odd heads' normalized output is DMA-shifted into the top half of a
    packed [128, cw] oT pair tile so the output projection contracts
    K=128 over a head pair in one matmul (engines cannot partition-shift,
    DMA can).
"""

import os
import sys

for _p in ("/opt/trn_rl_repo",):
    if _p not in sys.path and os.path.isdir(_p):
        sys.path.append(_p)

import numpy as np

import concourse.bass as bass
from concourse import bacc
import concourse.mybir as mybir
import concourse.tile as tile
from concourse.bass_utils import run_bass_kernel_spmd

F32 = mybir.dt.float32
F32R = mybir.dt.float32r
F16 = mybir.dt.float16

B, S, D = 2, 2048, 1280
HEADS = 20
HD = D // HEADS          # 64
N_CORES = 8
NH = (B * HEADS) // N_CORES  # heads per core = 5
P = 128


def r(ap):
    """View an fp32 AP as float32r for full-rate matmul."""
    return ap.bitcast(F32R)


def round_fp32r(x):
    """Round fp32 to the fp32r grid (11-bit mantissa, RNE) on the host."""
    u = np.ascontiguousarray(x, dtype=np.float32).view(np.uint32)
    lsb = (u >> 12) & 1
    u2 = (u + 0x7FF + lsb) & np.uint32(0xFFFFF000)
    return u2.view(np.float32)


def build_nc(s=S, d=D, nh=NH, hd=HD, cw=512):
    """Build the SPMD per-core program."""
    assert d % P == 0 and s % P == 0 and s % cw == 0 and cw % P == 0
    kt = d // P              # contraction tiles for projections
    c = nh * hd              # projection width
    n_cw = s // cw           # qi chunks
    n_kj = s // P            # key tiles
    st = s // P              # S tiles of 128
    n_pairs = nh // 2        # head pairs (2)
    sm_scale = 1.0 / float(np.sqrt(hd))

    nc = bacc.Bacc("TRN2", target_bir_lowering=False)
    hsT = nc.declare_dram_parameter("hsT", [d, s], F16, isOutput=False)
    wq = nc.declare_dram_parameter("wq", [d, c], F16, isOutput=False)
    wk = nc.declare_dram_parameter("wk", [d, c], F16, isOutput=False)
    wv = nc.declare_dram_parameter("wv", [d, c], F16, isOutput=False)
    wo = nc.declare_dram_parameter("wo", [c, d], F16, isOutput=False)
    y = nc.declare_dram_parameter("y", [s, d], F32, isOutput=True)

    hsT_t = hsT[:].rearrange("(ko p) s -> p ko s", p=P)   # [128, kt, s]
    wq_t = wq[:].rearrange("(ko p) c -> p ko c", p=P)
    wk_t = wk[:].rearrange("(ko p) c -> p ko c", p=P)
    wv_t = wv[:].rearrange("(ko p) c -> p ko c", p=P)

    # projection output column chunks (M <= 128)
    mchunks = [(i, min(i + P, c)) for i in range(0, c, P)]

    with tile.TileContext(nc) as tc:
        with tc.tile_pool(name="persist", bufs=1) as persist:
            # ---- persistent SBUF tensors ----
            # qT/kT packed two heads per 128-partition tile; the solo
            # head's tile is full-height with the head DUPLICATED in the
            # top half (same per-partition bytes, so no extra SBUF), which
            # lets its QK row-tile like the pairs: even kj from rows 0:64,
            # odd kj from rows 64:128, concurrently.
            n_qk_tiles = (c + P - 1) // P
            qT_tiles = [
                persist.tile([P, s], F16, name=f"qT{i}")
                for i in range(n_qk_tiles)
            ]
            kT_tiles = [
                persist.tile([P, s], F16, name=f"kT{i}")
                for i in range(n_qk_tiles)
            ]
            # v with ones column per head: [128, st, nh, hd+1]
            v_aug = persist.tile([P, st, nh, hd + 1], F16, name="v_aug")
            ones_f32 = persist.tile([P, 1], F32, name="ones_f32")
            nc.vector.memset(ones_f32[:], 1.0)
            # wo packed by head pair ([128, 2, d]: partitions 0:64 = even
            # head, 64:128 = odd head) plus the solo head's rows
            wo_pr = persist.tile([P, n_pairs, d], F16, name="wo_pr")
            wo_solo = persist.tile([hd, d], F16, name="wo_solo")
            # projection weights stay resident (qT chunks 1+ are produced
            # as filler during the attention phase).  DMA priority order:
            # wk and wv feed the first phase-1 matmuls.
            wq_sb = persist.tile([P, kt, c], F16, name="wq_sb")
            wk_sb = persist.tile([P, kt, c], F16, name="wk_sb")
            wv_sb = persist.tile([P, kt, c], F16, name="wv_sb")
            for k in range(kt):
                nc.sync.dma_start(out=wk_sb[:, k, :], in_=wk_t[:, k, :])
            for k in range(kt):
                nc.sync.dma_start(out=wv_sb[:, k, :], in_=wv_t[:, k, :])

            # ---- phase 1: kT (all chunks), v (all), qT chunk 0 ----
            with (
                tc.tile_pool(name="hstream", bufs=2) as hstream,
                tc.tile_pool(name="ps_proj", bufs=4, space="PSUM") as ps_proj,
            ):
                def hs_fetch(pncw):
                    t = hstream.tile([P, kt, cw], F16, name="hs_nc")
                    for k in range(kt):
                        nc.sync.dma_start(
                            out=t[:, k, :],
                            in_=hsT_t[:, k, pncw * cw : (pncw + 1) * cw],
                        )
                    return t

                # depth-2 prefetch: the DMA for chunk n+1 is issued while
                # chunk n computes (the bufs=2 pool orders buffer reuse),
                # so no chunk starts on a cold hidden-state fetch
                hs_q = [hs_fetch(0), hs_fetch(1)]
                for ncw in range(n_cw):
                    hs_nc = hs_q.pop(0)
                    if ncw + 2 < n_cw:
                        hs_q.append(hs_fetch(ncw + 2))
                    if ncw == 0:
                        # lower-priority weight DMAs queue behind the
                        # first hidden-state chunk
                        for k in range(kt):
                            nc.sync.dma_start(
                                out=wq_sb[:, k, :], in_=wq_t[:, k, :]
                            )
                        nc.sync.dma_start(
                            out=wo_pr[:],
                            in_=wo[0 : 2 * P, :].rearrange(
                                "(pr q) d -> q pr d", q=P
                            ),
                        )
                        nc.sync.dma_start(
                            out=wo_solo[:], in_=wo[2 * P : c, :]
                        )
                    srcs = [(wk_sb, kT_tiles)]
                    if ncw == 0:
                        srcs.append((wq_sb, qT_tiles))
                    cs = slice(ncw * cw, (ncw + 1) * cw)
                    for w_sb, dst_tiles in srcs:
                        for mi, (c0, c1) in enumerate(mchunks):
                            m = c1 - c0
                            ps_q = ps_proj.tile([m, cw], F32, tag="ps_q")
                            for k in range(kt):
                                nc.tensor.matmul(
                                    ps_q[:],
                                    w_sb[:, k, c0:c1],
                                    hs_nc[:, k, :],
                                    start=(k == 0),
                                    stop=(k == kt - 1),
                                )
                            with nc.allow_low_precision(reason="f16 qkT"):
                                nc.vector.tensor_copy(
                                    dst_tiles[mi][0:m, cs], ps_q[:]
                                )
                            if m < P:
                                # duplicate the solo head into the top
                                # half so its QK can row-tile (DMA does
                                # the partition shift an engine cannot)
                                nc.sync.dma_start(
                                    out=dst_tiles[mi][m:P, cs],
                                    in_=dst_tiles[mi][0:m, cs],
                                )
                    # v for the S-tiles inside this chunk
                    for ss in range(cw // P):
                        s_global = ncw * (cw // P) + ss
                        ps_v = ps_proj.tile([P, c], F32, tag="ps_v")
                        for k in range(kt):
                            nc.tensor.matmul(
                                ps_v[:],
                                hs_nc[:, k, ss * P : (ss + 1) * P],
                                wv_sb[:, k, :],
                                start=(k == 0),
                                stop=(k == kt - 1),
                            )
                        nc.any.tensor_copy(
                            v_aug[:, s_global, :, 0:hd],
                            ps_v[:].rearrange("p (h e) -> p h e", h=nh),
                        )
                        nc.any.tensor_copy(
                            v_aug[:, s_global, :, hd : hd + 1],
                            ones_f32[:].to_broadcast((P, nh, 1)),
                        )

            # ---- phase 2: attention with filler weave ----
            with (
                tc.tile_pool(name="ps_s", bufs=2, space="PSUM") as ps_s_pool,
                tc.tile_pool(name="ps_o", bufs=2, space="PSUM") as ps_o_pool,
                tc.tile_pool(name="ps_fill", bufs=2, space="PSUM") as ps_fill_pool,
                tc.tile_pool(name="hstream2", bufs=2) as hstream2,
                tc.tile_pool(name="exps", bufs=3) as exps_pool,
                tc.tile_pool(name="small", bufs=4) as small_pool,
                tc.tile_pool(name="otile", bufs=2) as otile_pool,
                tc.tile_pool(name="ystage", bufs=2) as ystage_pool,
            ):
                def emit_qk(u, ps_s, ncw):
                    kind, idx, g = u
                    if kind == "p":
                        # both heads of the pair, adjacent emission ->
                        # concurrent row-tiled execution
                        for half in range(2):
                            nc.tensor.matmul(
                                ps_s[:, half * cw : (half + 1) * cw],
                                kT_tiles[idx][half * hd : (half + 1) * hd,
                                              g * P : (g + 1) * P],
                                qT_tiles[idx][half * hd : (half + 1) * hd,
                                              ncw * cw : (ncw + 1) * cw],
                                start=True,
                                stop=True,
                            )
                    else:
                        # solo head: the duplicated tile halves let the
                        # two kj tiles run as a concurrent row-tiled pair
                        ht = nh // 2
                        for sl in range(2):
                            kj = 2 * g + sl
                            nc.tensor.matmul(
                                ps_s[:, sl * cw : (sl + 1) * cw],
                                kT_tiles[ht][sl * hd : (sl + 1) * hd,
                                             kj * P : (kj + 1) * P],
                                qT_tiles[ht][sl * hd : (sl + 1) * hd,
                                             ncw * cw : (ncw + 1) * cw],
                                start=True,
                                stop=True,
                            )

                def emit_pv(u, ps_oo, expS):
                    kind, idx, g = u
                    if kind == "p":
                        for half in range(2):
                            nc.tensor.matmul(
                                ps_oo[half][:],
                                v_aug[:, g, 2 * idx + half, :],
                                expS[:, half * cw : (half + 1) * cw],
                                start=(g == 0),
                                stop=(g == n_kj - 1),
                            )
                    else:
                        for sl in range(2):
                            kj = 2 * g + sl
                            nc.tensor.matmul(
                                ps_oo[0][:],
                                v_aug[:, kj, nh - 1, :],
                                expS[:, sl * cw : (sl + 1) * cw],
                                start=(kj == 0),
                                stop=(kj == n_kj - 1),
                            )

                def flush_copy(ps_o):
                    """Stage ps_o out of PSUM (releases the bank for the
                    next pair).  Custom DVE ops read partition 0 only, so
                    the denominator row gets its own partition-0 tile."""
                    o_un = small_pool.tile([hd, cw], F32, tag="o_un",
                                           name="o_un")
                    nc.vector.tensor_copy(o_un[:], ps_o[0:hd, :])
                    den = small_pool.tile([1, cw], F32, tag="den", name="den")
                    nc.vector.tensor_copy(den[:], ps_o[hd : hd + 1, :])
                    return o_un, den

                def flush_bcast(den):
                    """DVE approx-reciprocal then GpSimd broadcast across
                    the hd partitions.  The broadcast is the ONLY gpsimd
                    op in the kernel: mixing in gpsimd tensor ops forces a
                    ~7us LIBRARY_RELOAD per switch."""
                    rcp = small_pool.tile([1, cw], F32, tag="rcp", name="rcp")
                    nc.vector.reciprocal_approx_fast(rcp[:], den[:])
                    rcp_bc = small_pool.tile([hd, cw], F32, tag="rcp_bc",
                                             name="rcp_bc")
                    nc.gpsimd.partition_broadcast(rcp_bc[:], rcp[:])
                    return rcp_bc

                def flush_mul(o_un, rcp_bc, dst, dst_hi):
                    """dst_hi routes the odd head through a DMA
                    partition-shift into the top half of the packed pair
                    tile (engines cannot shift partitions, DMA can)."""
                    with nc.allow_low_precision(reason="f16 attn out"):
                        if dst_hi:
                            o_tmp = small_pool.tile([hd, cw], F16,
                                                    tag="o_tmp", name="o_tmp")
                            nc.vector.tensor_mul(o_tmp[:], o_un[:], rcp_bc[:])
                            nc.sync.dma_start(out=dst, in_=o_tmp[:])
                        else:
                            nc.vector.tensor_mul(dst, o_un[:], rcp_bc[:])

                # -- filler: single-matmul pieces fed between QK and PV --
                def qT_proj_pieces(pncw, hs2):
                    """qT projection for chunk pncw; the M=64 tail is
                    DMA-duplicated into the top half for solo-QK tiling."""
                    for mi, (c0, c1) in enumerate(mchunks):
                        m = c1 - c0
                        ps_q = ps_fill_pool.tile([m, cw], F32,
                                                 tag="fill", name="fill")

                        def mk(k, ps_q=ps_q, c0=c0, c1=c1, mi=mi, m=m):
                            def go():
                                nc.tensor.matmul(
                                    ps_q[:],
                                    wq_sb[:, k, c0:c1],
                                    hs2[:, k, :],
                                    start=(k == 0),
                                    stop=(k == kt - 1),
                                )
                                if k == kt - 1:
                                    pcs = slice(pncw * cw, (pncw + 1) * cw)
                                    with nc.allow_low_precision(reason="f16 q"):
                                        nc.vector.tensor_copy(
                                            qT_tiles[mi][0:m, pcs], ps_q[:]
                                        )
                                    if m < P:
                                        nc.sync.dma_start(
                                            out=qT_tiles[mi][m:P, pcs],
                                            in_=qT_tiles[mi][0:m, pcs],
                                        )
                            return go

                        for k in range(kt):
                            yield mk(k)

                def out_proj_pieces(oT_list, pncw):
                    """Output projection of chunk pncw: per seq-tile, per
                    d-chunk, a 3-matmul chain (two K=128 pair matmuls and
                    one K=64 solo) then a DVE drain."""
                    for tt in range(cw // P):
                        t_lo = (pncw * (cw // P) + tt) * P
                        tl = tt * P
                        y_sb = ystage_pool.tile([P, d], F32, tag="y_sb",
                                                name="y_sb")
                        for nn in range(0, d, 512):
                            ne = min(nn + 512, d)
                            ps_y = ps_fill_pool.tile([P, ne - nn], F32,
                                                     tag="fill", name="fill")

                            def mk(j, nn=nn, ne=ne, ps_y=ps_y, y_sb=y_sb,
                                   tl=tl, t_lo=t_lo):
                                def go():
                                    if j < n_pairs:
                                        nc.tensor.matmul(
                                            ps_y[:],
                                            oT_list[j][:, tl : tl + P],
                                            wo_pr[:, j, nn:ne],
                                            start=(j == 0),
                                            stop=False,
                                        )
                                    else:
                                        nc.tensor.matmul(
                                            ps_y[:],
                                            oT_list[j][:, tl : tl + P],
                                            wo_solo[:, nn:ne],
                                            start=False,
                                            stop=True,
                                        )
                                        nc.vector.tensor_copy(
                                            y_sb[:, nn:ne], ps_y[:]
                                        )
                                        if ne == d:
                                            nc.sync.dma_start(
                                                out=y[t_lo : t_lo + P, :],
                                                in_=y_sb[:],
                                            )
                                return go

                            for j in range(n_pairs + 1):
                                yield mk(j)

                # stage hidden-state slices for the filler qT projections
                # up front; DMAs execute during phase 1 / early attention
                hs2_tiles = {}
                for pncw in range(1, n_cw):
                    hs2 = hstream2.tile([P, kt, cw], F16, name="hs2")
                    for k in range(kt):
                        nc.sync.dma_start(
                            out=hs2[:, k, :],
                            in_=hsT_t[:, k, pncw * cw : (pncw + 1) * cw],
                        )
                    hs2_tiles[pncw] = hs2

                outT_by_chunk = [[] for _ in range(n_cw)]

                def chunk_filler(ncw):
                    def chain():
                        if ncw == 0:
                            for pncw in (1, 2):
                                yield from qT_proj_pieces(
                                    pncw, hs2_tiles[pncw]
                                )
                        else:
                            if ncw == 1:
                                yield from qT_proj_pieces(3, hs2_tiles[3])
                            yield from out_proj_pieces(
                                outT_by_chunk[ncw - 1], ncw - 1
                            )
                    return chain()

                def flush_done(u, ncw, ps_oo):
                    """At the last kj of a pair/solo, normalize into the
                    packed oT tiles used by the output projection.  All
                    PSUM-releasing copies go first."""
                    kind, idx, g = u
                    if kind == "p" and g == n_kj - 1:
                        oT_pair = otile_pool.tile([P, cw], F16,
                                                  tag=f"oTp{idx}", name="oTp")
                        staged = [flush_copy(ps_oo[0]), flush_copy(ps_oo[1])]
                        bcs = [flush_bcast(sd[1]) for sd in staged]
                        flush_mul(staged[0][0], bcs[0], oT_pair[0:hd, :],
                                  False)
                        flush_mul(staged[1][0], bcs[1], oT_pair[hd:P, :],
                                  True)
                        outT_by_chunk[ncw].append(oT_pair)
                    elif kind == "s" and g == n_kj // 2 - 1:
                        oT_solo = otile_pool.tile([hd, cw], F16, tag="oTs",
                                                  name="oTs")
                        o_un, den = flush_copy(ps_oo[0])
                        flush_mul(o_un, flush_bcast(den), oT_solo[:], False)
                        outT_by_chunk[ncw].append(oT_solo)

                # unit stream: per chunk, two head pairs (one kj tile per
                # unit) then the solo head (two kj tiles per unit)
                def chunk_units(ncw):
                    us = []
                    for pi in range(n_pairs):
                        us += [("p", pi, kj) for kj in range(n_kj)]
                    us += [("s", nh - 1, g) for g in range(n_kj // 2)]
                    return us

                stream = [
                    (ncw, u) for ncw in range(n_cw) for u in chunk_units(ncw)
                ]
                upc = len(chunk_units(0))  # units per chunk

                fillers = [None] * n_cw
                prev = None  # (u, ncw, ps_s, ps_oo)
                ps_oo = None
                for i, (ncw, u) in enumerate(stream):
                    kind, idx, g = u
                    if fillers[ncw] is None:
                        fillers[ncw] = chunk_filler(ncw)
                    ps_s = ps_s_pool.tile([P, 2 * cw], F32, tag="ps_s",
                                          name="ps_s")
                    emit_qk(u, ps_s, ncw)
                    if prev is not None:
                        p_u, p_ncw, p_ps_s, p_ps_oo = prev
                        fl = fillers[ncw]
                        # no filler on the first units of a chunk: the
                        # previous chunk's last flush chains (whose oT the
                        # out-proj filler reads) are still in flight, and
                        # a not-yet-ready filler piece blocks the in-order
                        # PE queue
                        in_chunk = i % upc
                        budget = 0 if in_chunk < 3 else (3 if (i % 2) else 2)
                        for _ in range(budget):
                            piece = next(fl, None)
                            if piece is None:
                                break
                            piece()
                        expS = exps_pool.tile([P, 2 * cw], F16, tag="expS",
                                              name="expS")
                        nc.scalar.activation(
                            expS[:], p_ps_s[:],
                            mybir.ActivationFunctionType.Exp,
                            scale=sm_scale,
                        )
                        emit_pv(p_u, p_ps_oo, expS)
                        flush_done(p_u, p_ncw, p_ps_oo)
                    if g == 0:
                        if kind == "p":
                            ps_oo = (
                                ps_o_pool.tile([hd + 1, cw], F32,
                                               tag="ps_o", name="ps_o"),
                                ps_o_pool.tile([hd + 1, cw], F32,
                                               tag="ps_o", name="ps_o"),
                            )
                        else:
                            ps_oo = (
                                ps_o_pool.tile([hd + 1, cw], F32,
                                               tag="ps_o", name="ps_o"),
                            )
                    prev = (u, ncw, ps_s, ps_oo)
                # tail
                p_u, p_ncw, p_ps_s, p_ps_oo = prev
                expS = exps_pool.tile([P, 2 * cw], F16, tag="expS",
                                      name="expS")
                nc.scalar.activation(
                    expS[:], p_ps_s[:],
                    mybir.ActivationFunctionType.Exp,
                    scale=sm_scale,
                )
                emit_pv(p_u, p_ps_oo, expS)
                flush_done(p_u, p_ncw, p_ps_oo)
                for fl in fillers:
                    if fl is not None:
                        for piece in fl:
                            piece()
                for piece in out_proj_pieces(outT_by_chunk[n_cw - 1],
                                             n_cw - 1):
                    piece()
    nc.compile()
    return nc


_NC_CACHE = {}


def _get_nc():
    key = (S, D, NH, HD)
    if key not in _NC_CACHE:
        _NC_CACHE[key] = build_nc()
    return _NC_CACHE[key]


def shard_inputs(hidden_states, Wq, Wk, Wv, Wo):
    """Build the 8 per-core input maps."""
    hs = np.asarray(hidden_states, dtype=np.float32)
    hsT = [np.ascontiguousarray(hs[b].T) for b in range(B)]  # [D, S] each
    Wo = np.asarray(Wo, dtype=np.float32)
    in_maps = []
    cores_per_b = N_CORES // B
    for core in range(N_CORES):
        b = core // cores_per_b
        h0 = (core % cores_per_b) * NH
        cols = slice(h0 * HD, (h0 + NH) * HD)
        in_maps.append(
            {
                "hsT": hsT[b].astype(np.float16),
                "wq": np.ascontiguousarray(np.asarray(Wq, np.float32)[:, cols]).astype(np.float16),
                "wk": np.ascontiguousarray(np.asarray(Wk, np.float32)[:, cols]).astype(np.float16),
                "wv": np.ascontiguousarray(np.asarray(Wv, np.float32)[:, cols]).astype(np.float16),
                "wo": np.ascontiguousarray(Wo[cols, :]).astype(np.float16),
            }
        )
    return in_maps


def kernel(hidden_states, Wq, Wk, Wv, Wo, bo, trace=False):
    nc = _get_nc()
    in_maps = shard_inputs(hidden_states, Wq, Wk, Wv, Wo)
    res = run_bass_kernel_spmd(
        nc, in_maps, core_ids=list(range(N_CORES)), trace=trace
    )
    cores_per_b = N_CORES // B
    out = np.empty((B, S, D), dtype=np.float32)
    bo32 = np.asarray(bo, dtype=np.float32)
    for b in range(B):
        acc = res.results[b * cores_per_b]["y"].astype(np.float32)
        for i in range(1, cores_per_b):
            acc = acc + res.results[b * cores_per_b + i]["y"]
        out[b] = acc + bo32
    if trace:
        kernel.last_exec_time_ns = res.exec_time_ns
        kernel.last_results = res
    return out


# revision 39
# speedup vs baseline: 1.0071x; 1.0071x over previous
"""Multi-head attention (AttnProcessor) Bass kernel for 8 Trainium2 cores.

Problem: hidden_states [2,2048,1280], Wq/Wk/Wv/Wo [1280,1280], bo [1280],
20 heads x head_dim 64.  out = softmax(q k^T / 8) v @ Wo + bo.

Sharding: 40 (batch, head) units -> 8 cores x 5 heads.  Cores 0-3 take
batch 0, cores 4-7 batch 1; each core gets a 5-head column slice of
Wq/Wk/Wv and the matching row slice of Wo, computes its partial output
projection [S, D], and the host sums the 4 partials per batch and adds bo.

Per-core design (the HAM clock gate drops the PE to 1.2 GHz whenever the
PE idles, so everything aims at a gap-free PE instruction stream):

  phase 1 (dense matmuls, no cross-engine deps):
    kT for the full sequence, v for the full sequence, qT for qi chunk 0.
  phase 2 (attention):
    Heads are processed in row-tiled PAIRS: head 2p lives in partitions
    0:64 of qT/kT tile p, head 2p+1 in 64:128.  Emitting both heads' QK
    for one key tile back-to-back makes the PE run them concurrently in
    different row-groups of the array (tile_position auto-derives from
    the base partition), so a K=64 QK costs the same as K=128.
    Per (pair, kj) unit: QK of the NEXT unit -> filler matmuls -> exp of
    this unit on ScalarE ([128, 1024] covering both heads) -> both PV
    matmuls.  The 5th head runs solo with two kj tiles per unit.
    Filler = qT projections for later chunks + the previous chunk's
    output projection, so the PE always has independent queued work and
    never waits on the ScalarE exp latency.
  scores are computed transposed (S^T = kT-slice x qT, K=hd) so PV needs
    no transpose and the ones-augmented V gives the softmax denominator
    in the same PSUM accumulation chain.
  normalization runs off the PE: DVE copies (o_un + denominator row) free
    the PSUM bank ~1us after the pair ends, then DVE
    reciprocal_approx_fast + GpSimd partition_broadcast + DVE multiply.
    Odd heads' normalized output is DMA-shifted into the top half of a
    packed [128, cw] oT pair tile so the output projection contracts
    K=128 over a head pair in one matmul (engines cannot partition-shift,
    DMA can).
"""

import os
import sys

for _p in ("/opt/trn_rl_repo",):
    if _p not in sys.path and os.path.isdir(_p):
        sys.path.append(_p)

import numpy as np

import concourse.bass as bass
from concourse import bacc
import concourse.mybir as mybir
import concourse.tile as tile
from concourse.bass_utils import run_bass_kernel_spmd

F32 = mybir.dt.float32
F32R = mybir.dt.float32r
F16 = mybir.dt.float16

B, S, D = 2, 2048, 1280
HEADS = 20
HD = D // HEADS          # 64
N_CORES = 8
NH = (B * HEADS) // N_CORES  # heads per core = 5
P = 128


def r(ap):
    """View an fp32 AP as float32r for full-rate matmul."""
    return ap.bitcast(F32R)


def round_fp32r(x):
    """Round fp32 to the fp32r grid (11-bit mantissa, RNE) on the host."""
    u = np.ascontiguousarray(x, dtype=np.float32).view(np.uint32)
    lsb = (u >> 12) & 1
    u2 = (u + 0x7FF + lsb) & np.uint32(0xFFFFF000)
    return u2.view(np.float32)


def build_nc(s=S, d=D, nh=NH, hd=HD, cw=512):
    """Build the SPMD per-core program."""
    assert d % P == 0 and s % P == 0 and s % cw == 0 and cw % P == 0
    kt = d // P              # contraction tiles for projections
    c = nh * hd              # projection width
    n_cw = s // cw           # qi chunks
    n_kj = s // P            # key tiles
    st = s // P              # S tiles of 128
    n_pairs = nh // 2        # head pairs (2)
    sm_scale = 1.0 / float(np.sqrt(hd))

    nc = bacc.Bacc("TRN2", target_bir_lowering=False)
    hsT = nc.declare_dram_parameter("hsT", [d, s], F16, isOutput=False)
    wq = nc.declare_dram_parameter("wq", [d, c], F16, isOutput=False)
    wk = nc.declare_dram_parameter("wk", [d, c], F16, isOutput=False)
    wv = nc.declare_dram_parameter("wv", [d, c], F16, isOutput=False)
    wo = nc.declare_dram_parameter("wo", [c, d], F16, isOutput=False)
    y = nc.declare_dram_parameter("y", [s, d], F32, isOutput=True)

    hsT_t = hsT[:].rearrange("(ko p) s -> p ko s", p=P)   # [128, kt, s]
    wq_t = wq[:].rearrange("(ko p) c -> p ko c", p=P)
    wk_t = wk[:].rearrange("(ko p) c -> p ko c", p=P)
    wv_t = wv[:].rearrange("(ko p) c -> p ko c", p=P)

    # projection output column chunks (M <= 128)
    mchunks = [(i, min(i + P, c)) for i in range(0, c, P)]

    with tile.TileContext(nc) as tc:
        with tc.tile_pool(name="persist", bufs=1) as persist:
            # ---- persistent SBUF tensors ----
            # qT/kT packed two heads per 128-partition tile; the solo
            # head's tile is full-height with the head DUPLICATED in the
            # top half (same per-partition bytes, so no extra SBUF), which
            # lets its QK row-tile like the pairs: even kj from rows 0:64,
            # odd kj from rows 64:128, concurrently.
            n_qk_tiles = (c + P - 1) // P
            qT_tiles = [
                persist.tile([P, s], F16, name=f"qT{i}")
                for i in range(n_qk_tiles)
            ]
            kT_tiles = [
                persist.tile([P, s], F16, name=f"kT{i}")
                for i in range(n_qk_tiles)
            ]
            # v with ones column per head: [128, st, nh, hd+1]
            v_aug = persist.tile([P, st, nh, hd + 1], F16, name="v_aug")
            ones_f32 = persist.tile([P, 1], F32, name="ones_f32")
            nc.vector.memset(ones_f32[:], 1.0)
            # wo packed by head pair ([128, 2, d]: partitions 0:64 = even
            # head, 64:128 = odd head) plus the solo head's rows
            wo_pr = persist.tile([P, n_pairs, d], F16, name="wo_pr")
            wo_solo = persist.tile([hd, d], F16, name="wo_solo")
            # projection weights stay resident (qT chunks 1+ are produced
            # as filler during the attention phase).  DMA priority order:
            # wk and wv feed the first phase-1 matmuls.
            wq_sb = persist.tile([P, kt, c], F16, name="wq_sb")
            wk_sb = persist.tile([P, kt, c], F16, name="wk_sb")
            wv_sb = persist.tile([P, kt, c], F16, name="wv_sb")
            for k in range(kt):
                nc.sync.dma_start(out=wk_sb[:, k, :], in_=wk_t[:, k, :])
            for k in range(kt):
                nc.sync.dma_start(out=wv_sb[:, k, :], in_=wv_t[:, k, :])

            # ---- phase 1: kT (all chunks), v (all), qT chunk 0 ----
            with (
                tc.tile_pool(name="hstream", bufs=2) as hstream,
                tc.tile_pool(name="ps_proj", bufs=4, space="PSUM") as ps_proj,
            ):
                for ncw in range(n_cw):
                    hs_nc = hstream.tile([P, kt, cw], F16, name="hs_nc")
                    for k in range(kt):
                        nc.sync.dma_start(
                            out=hs_nc[:, k, :],
                            in_=hsT_t[:, k, ncw * cw : (ncw + 1) * cw],
                        )
                    if ncw == 0:
                        # lower-priority weight DMAs queue behind the
                        # first hidden-state chunk
                        for k in range(kt):
                            nc.sync.dma_start(
                                out=wq_sb[:, k, :], in_=wq_t[:, k, :]
                            )
                        nc.sync.dma_start(
                            out=wo_pr[:],
                            in_=wo[0 : 2 * P, :].rearrange(
                                "(pr q) d -> q pr d", q=P
                            ),
                        )
                        nc.sync.dma_start(
                            out=wo_solo[:], in_=wo[2 * P : c, :]
                        )
                    srcs = [(wk_sb, kT_tiles)]
                    if ncw == 0:
                        srcs.append((wq_sb, qT_tiles))
                    cs = slice(ncw * cw, (ncw + 1) * cw)
                    for w_sb, dst_tiles in srcs:
                        for mi, (c0, c1) in enumerate(mchunks):
                            m = c1 - c0
                            ps_q = ps_proj.tile([m, cw], F32, tag="ps_q")
                            for k in range(kt):
                                nc.tensor.matmul(
                                    ps_q[:],
                                    w_sb[:, k, c0:c1],
                                    hs_nc[:, k, :],
                                    start=(k == 0),
                                    stop=(k == kt - 1),
                                )
                            with nc.allow_low_precision(reason="f16 qkT"):
                                nc.vector.tensor_copy(
                                    dst_tiles[mi][0:m, cs], ps_q[:]
                                )
                            if m < P:
                                # duplicate the solo head into the top
                                # half so its QK can row-tile (DMA does
                                # the partition shift an engine cannot)
                                nc.sync.dma_start(
                                    out=dst_tiles[mi][m:P, cs],
                                    in_=dst_tiles[mi][0:m, cs],
                                )
                    # v for the S-tiles inside this chunk
                    for ss in range(cw // P):
                        s_global = ncw * (cw // P) + ss
                        ps_v = ps_proj.tile([P, c], F32, tag="ps_v")
                        for k in range(kt):
                            nc.tensor.matmul(
                                ps_v[:],
                                hs_nc[:, k, ss * P : (ss + 1) * P],
                                wv_sb[:, k, :],
                                start=(k == 0),
                                stop=(k == kt - 1),
                            )
                        nc.any.tensor_copy(
                            v_aug[:, s_global, :, 0:hd],
                            ps_v[:].rearrange("p (h e) -> p h e", h=nh),
                        )
                        nc.any.tensor_copy(
                            v_aug[:, s_global, :, hd : hd + 1],
                            ones_f32[:].to_broadcast((P, nh, 1)),
                        )

            # ---- phase 2: attention with filler weave ----
            with (
                tc.tile_pool(name="ps_s", bufs=2, space="PSUM") as ps_s_pool,
                tc.tile_pool(name="ps_o", bufs=2, space="PSUM") as ps_o_pool,
                tc.tile_pool(name="ps_fill", bufs=2, space="PSUM") as ps_fill_pool,
                tc.tile_pool(name="hstream2", bufs=2) as hstream2,
                tc.tile_pool(name="exps", bufs=3) as exps_pool,
                tc.tile_pool(name="small", bufs=4) as small_pool,
                tc.tile_pool(name="otile", bufs=2) as otile_pool,
                tc.tile_pool(name="ystage", bufs=2) as ystage_pool,
            ):
                def emit_qk(u, ps_s, ncw):
                    kind, idx, g = u
                    if kind == "p":
                        # both heads of the pair, adjacent emission ->
                        # concurrent row-tiled execution
                        for half in range(2):
                            nc.tensor.matmul(
                                ps_s[:, half * cw : (half + 1) * cw],
                                kT_tiles[idx][half * hd : (half + 1) * hd,
                                              g * P : (g + 1) * P],
                                qT_tiles[idx][half * hd : (half + 1) * hd,
                                              ncw * cw : (ncw + 1) * cw],
                                start=True,
                                stop=True,
                            )
                    else:
                        # solo head: the duplicated tile halves let the
                        # two kj tiles run as a concurrent row-tiled pair
                        ht = nh // 2
                        for sl in range(2):
                            kj = 2 * g + sl
                            nc.tensor.matmul(
                                ps_s[:, sl * cw : (sl + 1) * cw],
                                kT_tiles[ht][sl * hd : (sl + 1) * hd,
                                             kj * P : (kj + 1) * P],
                                qT_tiles[ht][sl * hd : (sl + 1) * hd,
                                             ncw * cw : (ncw + 1) * cw],
                                start=True,
                                stop=True,
                            )

                def emit_pv(u, ps_oo, expS):
                    kind, idx, g = u
                    if kind == "p":
                        for half in range(2):
                            nc.tensor.matmul(
                                ps_oo[half][:],
                                v_aug[:, g, 2 * idx + half, :],
                                expS[:, half * cw : (half + 1) * cw],
                                start=(g == 0),
                                stop=(g == n_kj - 1),
                            )
                    else:
                        for sl in range(2):
                            kj = 2 * g + sl
                            nc.tensor.matmul(
                                ps_oo[0][:],
                                v_aug[:, kj, nh - 1, :],
                                expS[:, sl * cw : (sl + 1) * cw],
                                start=(kj == 0),
                                stop=(kj == n_kj - 1),
                            )

                def flush_copy(ps_o):
                    """Stage ps_o out of PSUM (releases the bank for the
                    next pair).  Custom DVE ops read partition 0 only, so
                    the denominator row gets its own partition-0 tile."""
                    o_un = small_pool.tile([hd, cw], F32, tag="o_un",
                                           name="o_un")
                    nc.vector.tensor_copy(o_un[:], ps_o[0:hd, :])
                    den = small_pool.tile([1, cw], F32, tag="den", name="den")
                    nc.vector.tensor_copy(den[:], ps_o[hd : hd + 1, :])
                    return o_un, den

                def flush_bcast(den):
                    """DVE approx-reciprocal then GpSimd broadcast across
                    the hd partitions.  The broadcast is the ONLY gpsimd
                    op in the kernel: mixing in gpsimd tensor ops forces a
                    ~7us LIBRARY_RELOAD per switch."""
                    rcp = small_pool.tile([1, cw], F32, tag="rcp", name="rcp")
                    nc.vector.reciprocal_approx_fast(rcp[:], den[:])
                    rcp_bc = small_pool.tile([hd, cw], F32, tag="rcp_bc",
                                             name="rcp_bc")
                    nc.gpsimd.partition_broadcast(rcp_bc[:], rcp[:])
                    return rcp_bc

                def flush_mul(o_un, rcp_bc, dst, dst_hi):
                    """dst_hi routes the odd head through a DMA
                    partition-shift into the top half of the packed pair
                    tile (engines cannot shift partitions, DMA can)."""
                    with nc.allow_low_precision(reason="f16 attn out"):
                        if dst_hi:
                            o_tmp = small_pool.tile([hd, cw], F16,
                                                    tag="o_tmp", name="o_tmp")
                            nc.vector.tensor_mul(o_tmp[:], o_un[:], rcp_bc[:])
                            nc.sync.dma_start(out=dst, in_=o_tmp[:])
                        else:
                            nc.vector.tensor_mul(dst, o_un[:], rcp_bc[:])

                # -- filler: single-matmul pieces fed between QK and PV --
                def qT_proj_pieces(pncw, hs2):
                    """qT projection for chunk pncw; the M=64 tail is
                    DMA-duplicated into the top half for solo-QK tiling."""
                    for mi, (c0, c1) in enumerate(mchunks):
                        m = c1 - c0
                        ps_q = ps_fill_pool.tile([m, cw], F32,
                                                 tag="fill", name="fill")

                        def mk(k, ps_q=ps_q, c0=c0, c1=c1, mi=mi, m=m):
                            def go():
                                nc.tensor.matmul(
                                    ps_q[:],
                                    wq_sb[:, k, c0:c1],
                                    hs2[:, k, :],
                                    start=(k == 0),
                                    stop=(k == kt - 1),
                                )
                                if k == kt - 1:
                                    pcs = slice(pncw * cw, (pncw + 1) * cw)
                                    with nc.allow_low_precision(reason="f16 q"):
                                        nc.vector.tensor_copy(
                                            qT_tiles[mi][0:m, pcs], ps_q[:]
                                        )
                                    if m < P:
                                        nc.sync.dma_start(
                                            out=qT_tiles[mi][m:P, pcs],
                                            in_=qT_tiles[mi][0:m, pcs],
                                        )
                            return go

                        for k in range(kt):
                            yield mk(k)

                def out_proj_pieces(oT_list, pncw):
                    """Output projection of chunk pncw: per seq-tile, per
                    d-chunk, a 3-matmul chain (two K=128 pair matmuls and
                    one K=64 solo) then a DVE drain."""
                    for tt in range(cw // P):
                        t_lo = (pncw * (cw // P) + tt) * P
                        tl = tt * P
                        y_sb = ystage_pool.tile([P, d], F32, tag="y_sb",
                                                name="y_sb")
                        for nn in range(0, d, 512):
                            ne = min(nn + 512, d)
                            ps_y = ps_fill_pool.tile([P, ne - nn], F32,
                                                     tag="fill", name="fill")

                            def mk(j, nn=nn, ne=ne, ps_y=ps_y, y_sb=y_sb,
                                   tl=tl, t_lo=t_lo):
                                def go():
                                    if j < n_pairs:
                                        nc.tensor.matmul(
                                            ps_y[:],
                                            oT_list[j][:, tl : tl + P],
                                            wo_pr[:, j, nn:ne],
                                            start=(j == 0),
                                            stop=False,
                                        )
                                    else:
                                        nc.tensor.matmul(
                                            ps_y[:],
                                            oT_list[j][:, tl : tl + P],
                                            wo_solo[:, nn:ne],
                                            start=False,
                                            stop=True,
                                        )
                                        nc.vector.tensor_copy(
                                            y_sb[:, nn:ne], ps_y[:]
                                        )
                                        if ne == d:
                                            nc.sync.dma_start(
                                                out=y[t_lo : t_lo + P, :],
                                                in_=y_sb[:],
                                            )
                                return go

                            for j in range(n_pairs + 1):
                                yield mk(j)

                # stage hidden-state slices for the filler qT projections
                # up front; DMAs execute during phase 1 / early attention
                hs2_tiles = {}
                for pncw in range(1, n_cw):
                    hs2 = hstream2.tile([P, kt, cw], F16, name="hs2")
                    for k in range(kt):
                        nc.sync.dma_start(
                            out=hs2[:, k, :],
                            in_=hsT_t[:, k, pncw * cw : (pncw + 1) * cw],
                        )
                    hs2_tiles[pncw] = hs2

                outT_by_chunk = [[] for _ in range(n_cw)]

                def chunk_filler(ncw):
                    def chain():
                        if ncw == 0:
                            for pncw in (1, 2):
                                yield from qT_proj_pieces(
                                    pncw, hs2_tiles[pncw]
                                )
                        else:
                            if ncw == 1:
                                yield from qT_proj_pieces(3, hs2_tiles[3])
                            yield from out_proj_pieces(
                                outT_by_chunk[ncw - 1], ncw - 1
                            )
                    return chain()

                def flush_done(u, ncw, ps_oo):
                    """At the last kj of a pair/solo, normalize into the
                    packed oT tiles used by the output projection.  All
                    PSUM-releasing copies go first."""
                    kind, idx, g = u
                    if kind == "p" and g == n_kj - 1:
                        oT_pair = otile_pool.tile([P, cw], F16,
                                                  tag=f"oTp{idx}", name="oTp")
                        staged = [flush_copy(ps_oo[0]), flush_copy(ps_oo[1])]
                        bcs = [flush_bcast(sd[1]) for sd in staged]
                        flush_mul(staged[0][0], bcs[0], oT_pair[0:hd, :],
                                  False)
                        flush_mul(staged[1][0], bcs[1], oT_pair[hd:P, :],
                                  True)
                        outT_by_chunk[ncw].append(oT_pair)
                    elif kind == "s" and g == n_kj // 2 - 1:
                        oT_solo = otile_pool.tile([hd, cw], F16, tag="oTs",
                                                  name="oTs")
                        o_un, den = flush_copy(ps_oo[0])
                        flush_mul(o_un, flush_bcast(den), oT_solo[:], False)
                        outT_by_chunk[ncw].append(oT_solo)

                # unit stream: per chunk, two head pairs (one kj tile per
                # unit) then the solo head (two kj tiles per unit)
                def chunk_units(ncw):
                    us = []
                    for pi in range(n_pairs):
                        us += [("p", pi, kj) for kj in range(n_kj)]
                    us += [("s", nh - 1, g) for g in range(n_kj // 2)]
                    return us

                stream = [
                    (ncw, u) for ncw in range(n_cw) for u in chunk_units(ncw)
                ]
                upc = len(chunk_units(0))  # units per chunk

                fillers = [None] * n_cw
                prev = None  # (u, ncw, ps_s, ps_oo)
                ps_oo = None
                for i, (ncw, u) in enumerate(stream):
                    kind, idx, g = u
                    if fillers[ncw] is None:
                        fillers[ncw] = chunk_filler(ncw)
                    ps_s = ps_s_pool.tile([P, 2 * cw], F32, tag="ps_s",
                                          name="ps_s")
                    emit_qk(u, ps_s, ncw)
                    if prev is not None:
                        p_u, p_ncw, p_ps_s, p_ps_oo = prev
                        fl = fillers[ncw]
                        # no filler on the first units of a chunk: the
                        # previous chunk's last flush chains (whose oT the
                        # out-proj filler reads) are still in flight, and
                        # a not-yet-ready filler piece blocks the in-order
                        # PE queue
                        in_chunk = i % upc
                        budget = 0 if in_chunk < 3 else (3 if (i % 2) else 2)
                        for _ in range(budget):
                            piece = next(fl, None)
                            if piece is None:
                                break
                            piece()
                        expS = exps_pool.tile([P, 2 * cw], F16, tag="expS",
                                              name="expS")
                        nc.scalar.activation(
                            expS[:], p_ps_s[:],
                            mybir.ActivationFunctionType.Exp,
                            scale=sm_scale,
                        )
                        emit_pv(p_u, p_ps_oo, expS)
                        flush_done(p_u, p_ncw, p_ps_oo)
                    if g == 0:
                        if kind == "p":
                            ps_oo = (
                                ps_o_pool.tile([hd + 1, cw], F32,
                                               tag="ps_o", name="ps_o"),
                                ps_o_pool.tile([hd + 1, cw], F32,
                                               tag="ps_o", name="ps_o"),
                            )
                        else:
                            ps_oo = (
                                ps_o_pool.tile([hd + 1, cw], F32,
                                               tag="ps_o", name="ps_o"),
                            )
                    prev = (u, ncw, ps_s, ps_oo)
                # tail
                p_u, p_ncw, p_ps_s, p_ps_oo = prev
                expS = exps_pool.tile([P, 2 * cw], F16, tag="expS",
                                      name="expS")
                nc.scalar.activation(
                    expS[:], p_ps_s[:],
                    mybir.ActivationFunctionType.Exp,
                    scale=sm_scale,
                )
                emit_pv(p_u, p_ps_oo, expS)
                flush_done(p_u, p_ncw, p_ps_oo)
                for fl in fillers:
                    if fl is not None:
                        for piece in fl:
                            piece()
                for piece in out_proj_pieces(outT_by_chunk[n_cw - 1],
                                             n_cw - 1):
                    piece()
    nc.compile()
    return nc


_NC_CACHE = {}


def _get_nc():
    key = (S, D, NH, HD)
    if key not in _NC_CACHE:
        _NC_CACHE[key] = build_nc()
    return _NC_CACHE[key]


def shard_inputs(hidden_states, Wq, Wk, Wv, Wo):
    """Build the 8 per-core input maps."""
    hs = np.asarray(hidden_states, dtype=np.float32)
    hsT = [np.ascontiguousarray(hs[b].T) for b in range(B)]  # [D, S] each
    Wo = np.asarray(Wo, dtype=np.float32)
    in_maps = []
    cores_per_b = N_CORES // B
    for core in range(N_CORES):
        b = core // cores_per_b
        h0 = (core % cores_per_b) * NH
        cols = slice(h0 * HD, (h0 + NH) * HD)
        in_maps.append(
            {
                "hsT": hsT[b].astype(np.float16),
                "wq": np.ascontiguousarray(np.asarray(Wq, np.float32)[:, cols]).astype(np.float16),
                "wk": np.ascontiguousarray(np.asarray(Wk, np.float32)[:, cols]).astype(np.float16),
                "wv": np.ascontiguousarray(np.asarray(Wv, np.float32)[:, cols]).astype(np.float16),
                "wo": np.ascontiguousarray(Wo[cols, :]).astype(np.float16),
            }
        )
    return in_maps


def kernel(hidden_states, Wq, Wk, Wv, Wo, bo, trace=False):
    nc = _get_nc()
    in_maps = shard_inputs(hidden_states, Wq, Wk, Wv, Wo)
    res = run_bass_kernel_spmd(
        nc, in_maps, core_ids=list(range(N_CORES)), trace=trace
    )
    cores_per_b = N_CORES // B
    out = np.empty((B, S, D), dtype=np.float32)
    bo32 = np.asarray(bo, dtype=np.float32)
    for b in range(B):
        acc = res.results[b * cores_per_b]["y"].astype(np.float32)
        for i in range(1, cores_per_b):
            acc = acc + res.results[b * cores_per_b + i]["y"]
        out[b] = acc + bo32
    if trace:
        kernel.last_exec_time_ns = res.exec_time_ns
        kernel.last_results = res
    return out
